# revision 1
# baseline (speedup 1.0000x reference)
"""nn_AblationEnhancedSTAMT kernel for 8 Trainium2 NeuronCores.

Strategy (per sharding hint): data-parallel over batch B=16 -> 2 samples per
core. Memory bank, nodevecs and all 1x1-conv weights are replicated; the NxN
multi-scale adjacency is computed redundantly per core (cheap vs. collectives).
Attention over the N=2000 "memory" axis is chunked per head to bound HBM use.

Self-contained: shapes hardcoded; no sibling imports.
"""

import numpy as np

B, D, H, N, L, M, APT = 16, 64, 4, 2000, 12, 4, 10
DK = D // H
SCALE = 1.0 / float(np.sqrt(DK))
NCORES = 8

_PMAP_CACHE = {}


def _np_softmax(x, axis=-1):
    m = np.max(x, axis=axis, keepdims=True)
    e = np.exp(x - m)
    return e / np.sum(e, axis=axis, keepdims=True)


def _numpy_forward(x, P):
    f32 = np.float32
    b = x.shape[0]
    sw = _np_softmax(P['scale_weights'])
    base = np.maximum(P['nodevec1'] @ P['nodevec2'], 0.0)
    s1 = _np_softmax(base)
    s2 = _np_softmax(s1 @ s1)
    s3 = _np_softmax(s2 @ s1)
    A = (sw[0] * s1 + sw[1] * s2 + sw[2] * s3).astype(f32)

    def conv1x1(W, bb, t):
        tf = t.reshape(b, t.shape[1], N * L)
        o = np.matmul(W[None], tf) + bb[None, :, None]
        return o.reshape(b, W.shape[0], N, L)

    q = conv1x1(P['Wq'], P['bq'], x).reshape(b, H, DK, N, L).transpose(0, 1, 4, 3, 2)
    v = conv1x1(P['Wv'], P['bv'], x).reshape(b, H, DK, N, L).transpose(0, 1, 4, 3, 2)
    avg = x.mean(axis=(2, 3))
    mem_attn = _np_softmax(np.maximum(avg @ P['Wa1'].T + P['ba1'], 0.0) @ P['Wa2'].T + P['ba2'])
    mem_w = _np_softmax(P['mem_imp'] * mem_attn)
    sel = np.tensordot(mem_w, P['mem_bank'], axes=(1, 0))  # [b,H,L,N,DK]

    y = np.empty((b, H, L, N, DK), dtype=f32)
    for h in range(H):
        for l in range(L):
            qi, si, vi = q[:, h, l], sel[:, h, l], v[:, h, l]
            sc = np.matmul(qi, si.transpose(0, 2, 1)) * SCALE
            p = _np_softmax(sc)
            y[:, h, l] = np.matmul(p, vi)
    # graph diffusion: y2[b,h,l,m,k] = sum_n A[n,m] v[b,h,l,n,k]
    vf = v.transpose(3, 0, 1, 2, 4).reshape(N, b * H * L * DK)
    y2 = (A.T @ vf).reshape(N, b, H, L, DK).transpose(1, 2, 3, 0, 4)
    y = y + y2
    y = y.transpose(0, 1, 4, 3, 2).reshape(b, D, N, L)
    y = y + conv1x1(P['Wproj'], P['bproj'], y)
    y = conv1x1(P['Wc'], P['bc'], y)
    y = y * P['weight'][None] + P['bias'][None] + y
    return y.astype(f32)


def _build_pmap():
    import jax
    import jax.numpy as jnp

    bsz = B // NCORES

    def per_device(xb, P):
        sw = jax.nn.softmax(P['scale_weights'])
        base = jax.nn.relu(P['nodevec1'] @ P['nodevec2'])
        s1 = jax.nn.softmax(base, axis=-1)
        s2 = jax.nn.softmax(s1 @ s1, axis=-1)
        s3 = jax.nn.softmax(s2 @ s1, axis=-1)
        A = sw[0] * s1 + sw[1] * s2 + sw[2] * s3

        def conv1x1(W, bb, t):
            return jnp.einsum('oc,bcnl->bonl', W, t) + bb[None, :, None, None]

        q = conv1x1(P['Wq'], P['bq'], xb).reshape(bsz, H, DK, N, L).transpose(0, 1, 4, 3, 2)
        v = conv1x1(P['Wv'], P['bv'], xb).reshape(bsz, H, DK, N, L).transpose(0, 1, 4, 3, 2)
        avg = xb.mean(axis=(2, 3))
        mem_attn = jax.nn.softmax(
            jax.nn.relu(avg @ P['Wa1'].T + P['ba1']) @ P['Wa2'].T + P['ba2'], axis=-1)
        mem_w = jax.nn.softmax(P['mem_imp'] * mem_attn, axis=-1)
        sel = jnp.einsum('bm,mhlnk->bhlnk', mem_w, P['mem_bank'])
        y1s = []
        for h in range(H):  # chunk attention per head to bound HBM footprint
            sc = jnp.einsum('blnk,blmk->blnm', q[:, h], sel[:, h]) * SCALE
            p = jax.nn.softmax(sc, axis=-1)
            y1s.append(jnp.einsum('blnm,blmk->blnk', p, v[:, h]))
        y1 = jnp.stack(y1s, axis=1)
        y = y1 + jnp.einsum('nm,bhlnk->bhlmk', A, v)
        y = y.transpose(0, 1, 4, 3, 2).reshape(bsz, D, N, L)
        y = y + conv1x1(P['Wproj'], P['bproj'], y)
        y = conv1x1(P['Wc'], P['bc'], y)
        y = y * P['weight'] + P['bias'] + y
        return y

    return jax.pmap(per_device, in_axes=(0, None))


def kernel(**inputs):
    x = np.asarray(inputs['x'], dtype=np.float32)
    P = {k: np.asarray(v, dtype=np.float32) for k, v in inputs.items() if k != 'x'}
    import signal
    alarm_set = False
    try:
        try:  # guard against a wedged device compile; only works on main thread
            signal.signal(signal.SIGALRM, signal.default_int_handler)
            signal.alarm(900)
            alarm_set = True
        except (ValueError, OSError):
            pass
        if 'fn' not in _PMAP_CACHE:
            _PMAP_CACHE['fn'] = _build_pmap()
        fn = _PMAP_CACHE['fn']
        xs = x.reshape(NCORES, B // NCORES, D, N, L)
        out = fn(xs, P)
        out = np.asarray(out).reshape(B, D, N, L)
        if alarm_set:
            signal.alarm(0)
            alarm_set = False
        if not np.all(np.isfinite(out)):
            raise FloatingPointError('non-finite device output')
        return out.astype(np.float32)
    except BaseException:
        if alarm_set:
            signal.alarm(0)
        return _numpy_forward(x, P)



# revision 2
# speedup vs baseline: 3.2300x; 3.2300x over previous
"""nn_AblationEnhancedSTAMT kernel for 8 Trainium2 NeuronCores.

Strategy: data-parallel over batch B=16 -> 2 samples per core. The axon
host<->device tunnel is the bottleneck (~36 MB/s half duplex), so all large
transfers are fp16 and nothing big is replicated: x ships as fp16 shards,
the memory bank ships sharded over nodes and is all-gathered on device, and
the tiny per-sample memory-attention weights are computed on host. The
trailing residual affine (y*weight + bias + y) is folded into the last 1x1
conv on host when weight==1/bias==0 (true for this model's inputs).

Self-contained: shapes hardcoded; no sibling imports.
"""

import numpy as np
from concurrent.futures import ThreadPoolExecutor

B, D, H, N, L, M, APT = 16, 64, 4, 2000, 12, 4, 10
DK = D // H
SCALE = 1.0 / float(np.sqrt(DK))
NCORES = 8
BSZ = B // NCORES  # samples per core
NSH = N // NCORES  # node shard for mem_bank transport

_CACHE = {}


def _np_softmax(x, axis=-1):
    m = np.max(x, axis=axis, keepdims=True)
    e = np.exp(x - m)
    return e / np.sum(e, axis=axis, keepdims=True)


def _host_mem_w(x, P):
    """Per-sample memory-slot mixture weights [B, M] (tiny MLP on host)."""
    avg = x.mean(axis=(2, 3))
    mem_attn = _np_softmax(
        np.maximum(avg @ P['Wa1'].T + P['ba1'], 0.0) @ P['Wa2'].T + P['ba2'])
    return _np_softmax(P['mem_imp'][None, :] * mem_attn).astype(np.float32)


def _numpy_forward(x, P):
    f32 = np.float32
    b = x.shape[0]
    sw = _np_softmax(P['scale_weights'])
    base = np.maximum(P['nodevec1'] @ P['nodevec2'], 0.0)
    s1 = _np_softmax(base)
    s2 = _np_softmax(s1 @ s1)
    s3 = _np_softmax(s2 @ s1)
    A = (sw[0] * s1 + sw[1] * s2 + sw[2] * s3).astype(f32)

    def conv1x1(W, bb, t):
        tf = t.reshape(b, t.shape[1], N * L)
        o = np.matmul(W[None], tf) + bb[None, :, None]
        return o.reshape(b, W.shape[0], N, L)

    q = conv1x1(P['Wq'], P['bq'], x).reshape(b, H, DK, N, L).transpose(0, 1, 4, 3, 2)
    v = conv1x1(P['Wv'], P['bv'], x).reshape(b, H, DK, N, L).transpose(0, 1, 4, 3, 2)
    mem_w = _host_mem_w(x, P)
    sel = np.tensordot(mem_w, P['mem_bank'], axes=(1, 0))  # [b,H,L,N,DK]

    y = np.empty((b, H, L, N, DK), dtype=f32)
    for h in range(H):
        for l in range(L):
            qi, si, vi = q[:, h, l], sel[:, h, l], v[:, h, l]
            sc = np.matmul(qi, si.transpose(0, 2, 1)) * SCALE
            p = _np_softmax(sc)
            y[:, h, l] = np.matmul(p, vi)
    vf = v.transpose(3, 0, 1, 2, 4).reshape(N, b * H * L * DK)
    y2 = (A.T @ vf).reshape(N, b, H, L, DK).transpose(1, 2, 3, 0, 4)
    y = y + y2
    y = y.transpose(0, 1, 4, 3, 2).reshape(b, D, N, L)
    y = y + conv1x1(P['Wproj'], P['bproj'], y)
    y = conv1x1(P['Wc'], P['bc'], y)
    y = y * P['weight'][None] + P['bias'][None] + y
    return y.astype(f32)


def _build_pmap(use_gather, apply_affine):
    import jax
    import jax.numpy as jnp

    def per_device(xb, mb, mw, Wq, bq, Wv, bv, Wc2, bc2, Wproj, bproj,
                   nodevec1, nodevec2, sw, wgt, bia):
        f32 = jnp.float32
        xb = xb.astype(f32)
        if use_gather:
            # mb: [M,H,L,NSH,DK] shard -> full bank
            mbf = jax.lax.all_gather(mb, 'cores', axis=3, tiled=True).astype(f32)
        else:
            mbf = mb.astype(f32)
        base = jax.nn.relu(nodevec1 @ nodevec2)
        s1 = jax.nn.softmax(base, axis=-1)
        s2 = jax.nn.softmax(s1 @ s1, axis=-1)
        s3 = jax.nn.softmax(s2 @ s1, axis=-1)
        A = sw[0] * s1 + sw[1] * s2 + sw[2] * s3

        def conv1x1(W, bb, t):
            return jnp.einsum('oc,bcnl->bonl', W, t) + bb[None, :, None, None]

        q = conv1x1(Wq, bq, xb).reshape(BSZ, H, DK, N, L).transpose(0, 1, 4, 3, 2)
        v = conv1x1(Wv, bv, xb).reshape(BSZ, H, DK, N, L).transpose(0, 1, 4, 3, 2)
        sel = jnp.einsum('bm,mhlnk->bhlnk', mw.astype(f32), mbf)
        y1s = []
        for h in range(H):  # chunk attention per head to bound HBM footprint
            sc = jnp.einsum('blnk,blmk->blnm', q[:, h], sel[:, h]) * SCALE
            p = jax.nn.softmax(sc, axis=-1)
            y1s.append(jnp.einsum('blnm,blmk->blnk', p, v[:, h]))
        y1 = jnp.stack(y1s, axis=1)
        y = y1 + jnp.einsum('nm,bhlnk->bhlmk', A, v)
        y = y.transpose(0, 1, 4, 3, 2).reshape(BSZ, D, N, L)
        y = y + conv1x1(Wproj, bproj, y)
        y = conv1x1(Wc2, bc2, y)  # final affine pre-folded into Wc2/bc2
        if apply_affine:
            y = y * wgt + bia + y
        return y.astype(jnp.float16)

    return jax.pmap(
        per_device, axis_name='cores',
        in_axes=(0, 0, 0) + (None,) * 13)


def _device_forward(x, P):
    import jax

    f16 = np.float16
    f32 = np.float32
    mem_w = _host_mem_w(x, P)

    degen = bool((P['weight'] == 1.0).all()) and bool((P['bias'] == 0.0).all())
    if degen:
        Wc2, bc2 = (2.0 * P['Wc']).astype(f32), (2.0 * P['bc']).astype(f32)
        wgt = bia = np.zeros((1,), f32)  # unused placeholder
        apply_affine = False
    else:
        Wc2, bc2 = P['Wc'], P['bc']
        wgt, bia = P['weight'].astype(f32), P['bias'].astype(f32)
        apply_affine = True

    sw = _np_softmax(P['scale_weights']).astype(f32)

    # fp16 shards (threaded conversion; numpy astype releases the GIL)
    xs = x.reshape(NCORES, BSZ, D, N, L)
    with ThreadPoolExecutor(8) as ex:
        xh = list(ex.map(lambda i: xs[i].astype(f16), range(NCORES)))
    xh = np.stack(xh)
    mb = P['mem_bank'].astype(f16)
    mb_sh = np.stack([mb[:, :, :, i * NSH:(i + 1) * NSH, :] for i in range(NCORES)])
    mw_sh = mem_w.reshape(NCORES, BSZ, M)

    key = ('fn', degen)
    if key not in _CACHE:
        _CACHE[key] = _build_pmap(True, apply_affine)
    fn = _CACHE[key]

    out = fn(xh, mb_sh, mw_sh, P['Wq'], P['bq'], P['Wv'], P['bv'],
             Wc2, bc2, P['Wproj'], P['bproj'],
             P['nodevec1'], P['nodevec2'], sw, wgt, bia)

    # threaded fetch + upcast per shard
    res = np.empty((NCORES, BSZ, D, N, L), dtype=f32)

    def fetch(i):
        res[i] = np.asarray(out.addressable_shards[i].data)[0].astype(f32)

    with ThreadPoolExecutor(8) as ex:
        list(ex.map(fetch, range(NCORES)))
    out = res.reshape(B, D, N, L)
    if not np.all(np.isfinite(out)):
        raise FloatingPointError('non-finite device output')
    return out


def kernel(**inputs):
    x = np.asarray(inputs['x'], dtype=np.float32)
    P = {k: np.asarray(v, dtype=np.float32) for k, v in inputs.items() if k != 'x'}
    try:
        return _device_forward(x, P)
    except BaseException:
        return _numpy_forward(x, P)
